# revision 1
# baseline (speedup 1.0000x reference)
"""Fused additive-attention kernel for Trainium2 (8 NeuronCores, SPMD).

Computes  w = softmax_K( mask ? (Wl . tanh(vW_v^T + qW_q^T) + bl) : -1e9 )
without ever materializing the [B,N,S,K,H] joint_repr intermediate.

Sharding: data-parallel over batch B (16) across 8 cores -> 2 batches/core.
Weights replicated. Host does layout prep only (transposes / packing); all
FLOPs (matmuls, tanh, softmax) run on device.

Per-core dataflow (h on partitions for the hot loop), phased per h-chunk so
compute starts before all weights arrive:
  qpT[hc] [128(h), 512(b,ns)] = WqT-slice.T @ qT        (PE, psum acc, bf16)
  vpT     duplicated-pair table VP2[p, 2k]=VP2[p, 2k+1] (so the broadcast add
          below can use an inner step-1 pair AP -> DVE 2x_1P mode)
  JT      [128, (kk,b,ns)] bf16 = qpT + vp[b,k]         (DVE tensor_tensor,
          vp read via [[0,128],[1,2]] broadcast AP)
  tanh in-place on JT (one big ACT op per (hc, k-group))
  logits  psum [57, 512]: rows 0:25 = k<25, rows 32:57 = k>=25, accumulated
          with zero-padded Wl lhsT; (k, k+25) share lhsT and run as adjacent
          matmuls on PSUM col-strips 0/32 (tile_position col-tiling).
  masked softmax over k after PE-transposing logits to [ns, k].
"""

import os
import sys

import numpy as np

sys.path.insert(0, "/opt/trn_rl_repo")

import concourse.bass as bass
import concourse.mybir as mybir
from concourse import bacc, bass_utils
from concourse.tile import TileContext

# Problem shapes (hardcoded per contract -- kernel.py must be self-contained)
B, N, S, K = 16, 4, 64, 50
VD, QD, H = 1024, 768, 512
NCORES = 8
BPC = B // NCORES          # batches per core = 2
NS = BPC * N * S           # 512 rows (b, n, s) per core
KB = BPC * K               # 100 (b, k) columns per core
HC = H // 128              # 4 h-chunks
QC = QD // 128             # 6 qd-chunks
VC = VD // 128             # 8 vd-chunks

# j-groups per h-chunk: lists of Wl-variant indices j (k = j and k = j + 25
# are processed together, sharing the lhsT).  hc0 ramps up with a small first
# group so the first tanh issues as early as possible.
GROUPS_HC0 = [list(range(0, 5)), list(range(5, 15)), list(range(15, 25))]
GROUPS_HCX = [list(range(0, 10)), list(range(10, 20)), list(range(20, 25))]

F32 = mybir.dt.float32
BF16 = mybir.dt.bfloat16

_CACHE = {}


def _build_nc():
    nc = bacc.Bacc("TRN2", target_bir_lowering=False)

    qT_h = nc.dram_tensor("qT", [QD, NS], BF16, kind="ExternalInput")
    vT_h = nc.dram_tensor("vT", [VD, KB], BF16, kind="ExternalInput")
    # weight slabs, pre-split by hc-pair: A = h cols 0:256, B = 256:512
    WqTA_h = nc.dram_tensor("WqTA", [QD, 256], BF16, kind="ExternalInput")
    WqTB_h = nc.dram_tensor("WqTB", [QD, 256], BF16, kind="ExternalInput")
    WvTA_h = nc.dram_tensor("WvTA", [VD, 256], BF16, kind="ExternalInput")
    WvTB_h = nc.dram_tensor("WvTB", [VD, 256], BF16, kind="ExternalInput")
    # packed [128, 12]: cols 0:4 Wl chunks, 4:8 bq chunks, 8:12 bv chunks
    wlb_h = nc.dram_tensor("wlb", [128, 12], F32, kind="ExternalInput")
    # zero-padded Wl variants: [128, hc*625 + j*25 + c] = Wl[hc*128+p]*(c==j)
    wlz_h = nc.dram_tensor("wlz", [128, HC * 25 * 25], BF16, kind="ExternalInput")
    # packed [128, 200]: cols 0:100 maskf (b,k) replicated, 100:200 (maskf-1)*1e9
    msk_h = nc.dram_tensor("msk", [128, 2 * KB], F32, kind="ExternalInput")
    id_h = nc.dram_tensor("ident", [128, 128], F32, kind="ExternalInput")
    out_h = nc.dram_tensor("out", [NS, K], F32, kind="ExternalOutput")

    with TileContext(nc) as tc:
        with (
            tc.tile_pool(name="persist", bufs=1) as pp,
            tc.tile_pool(name="ppsum", bufs=1, space="PSUM") as ppsum,
            tc.tile_pool(name="smpsum", bufs=2, space="PSUM") as sps,
        ):
            # ---- DMA loads, chunked + ordered so the hc0 projection chain
            # starts after the first (qts, wqtA) chunk instead of all loads ----
            vts = pp.tile([128, VC, KB], BF16, name="vts")
            nc.sync.dma_start(
                vts[:, :, :], vT_h[:, :].rearrange("(c p) j -> p c j", p=128)
            )
            qts = pp.tile([128, QC, NS], BF16, name="qts")
            wqtA = pp.tile([128, QC, 256], BF16, name="wqtA")
            qT_r = qT_h[:, :].rearrange("(c p) j -> p c j", p=128)
            wqA_r = WqTA_h[:, :].rearrange("(c p) j -> p c j", p=128)
            for c in range(3):
                nc.sync.dma_start(
                    qts[:, 2 * c : 2 * c + 2, :], qT_r[:, 2 * c : 2 * c + 2, :]
                )
                nc.sync.dma_start(
                    wqtA[:, 2 * c : 2 * c + 2, :], wqA_r[:, 2 * c : 2 * c + 2, :]
                )
                if c == 0:
                    wvtA = pp.tile([128, VC, 256], BF16, name="wvtA")
                    nc.sync.dma_start(
                        wvtA[:, :, :],
                        WvTA_h[:, :].rearrange("(c p) j -> p c j", p=128),
                    )
            wlb = pp.tile_from(wlb_h[:, :], name="wlb")
            wlz = pp.tile_from(wlz_h[:, :], name="wlz")
            msk = pp.tile_from(msk_h[:, :], name="msk")
            ident = pp.tile_from(id_h[:, :], name="ident")
            wqtB = pp.tile([128, QC, 256], BF16, name="wqtB")
            nc.sync.dma_start(
                wqtB[:, :, :], WqTB_h[:, :].rearrange("(c p) j -> p c j", p=128)
            )
            wvtB = pp.tile([128, VC, 256], BF16, name="wvtB")
            nc.sync.dma_start(
                wvtB[:, :, :], WvTB_h[:, :].rearrange("(c p) j -> p c j", p=128)
            )

            # qpT (all h-chunks): [128, (hc, b, ns)] bf16
            QPs = pp.tile([128, HC * NS], BF16, name="QPs")
            # duplicated-pair vp table: [128, (hc, b, k, 2)] bf16
            VP2 = pp.tile([128, HC * KB * 2], BF16, name="VP2")

            # logits psum [57, 512]: rows 0:25 <- k 0:25 (col strip 0),
            # rows 32:57 <- k 25:50 (col strip 32)
            ps_log = ppsum.tile([57, NS], F32, name="ps_log")

            def proj_phase(ph, wqt, wvt):
                """Compute QPs/VP2 h-chunks [2*ph, 2*ph+2) from slab wqt/wvt."""
                with tc.tile_pool(name=f"p1ps{ph}", bufs=2, space="PSUM") as p1ps:
                    for i in range(2):
                        hc = 2 * ph + i
                        pq = p1ps.tile([128, NS], F32, tag="pq", name="pq")
                        for qc in range(QC):
                            nc.tensor.matmul(
                                pq[:, :],
                                wqt[:, qc, i * 128 : (i + 1) * 128],
                                qts[:, qc, :],
                                start=(qc == 0),
                                stop=(qc == QC - 1),
                            )
                        pv = p1ps.tile([128, KB], F32, tag="pv", name="pv")
                        for vc in range(VC):
                            nc.tensor.matmul(
                                pv[:, :],
                                wvt[:, vc, i * 128 : (i + 1) * 128],
                                vts[:, vc, :],
                                start=(vc == 0),
                                stop=(vc == VC - 1),
                            )
                        nc.vector.tensor_scalar_add(
                            QPs[:, hc * NS : (hc + 1) * NS],
                            pq[:, :],
                            wlb[:, HC + hc : HC + hc + 1],
                        )
                        vp2v = VP2[
                            :, hc * 2 * KB : (hc + 1) * 2 * KB
                        ].rearrange("p (k two) -> p k two", two=2)
                        pv3 = pv[:, :].rearrange("p (k one) -> p k one", one=1)
                        for par in range(2):
                            nc.vector.tensor_scalar_add(
                                vp2v[:, :, par : par + 1],
                                pv3[:, :, :],
                                wlb[:, 2 * HC + hc : 2 * HC + hc + 1],
                            )

            def main_hc(hc, mp, mid_cb=None):
                """Joint tanh + logit matmuls for one h-chunk."""
                groups = GROUPS_HC0 if hc == 0 else GROUPS_HCX
                for g, js in enumerate(groups):
                    if g == 1 and mid_cb is not None:
                        mid_cb()
                    L = len(js)
                    JT = mp.tile([128, 2 * L * NS], BF16, tag="JT", name="JT")
                    for kk in range(2 * L):
                        k = js[kk] if kk < L else js[kk - L] + 25
                        for b in range(BPC):
                            off = kk * NS + b * (NS // BPC)
                            c2 = hc * 2 * KB + (b * K + k) * 2
                            nc.vector.tensor_add(
                                JT[:, off : off + NS // BPC].rearrange(
                                    "p (x c) -> p x c", c=2
                                ),
                                QPs[
                                    :,
                                    hc * NS
                                    + b * (NS // BPC) : hc * NS
                                    + (b + 1) * (NS // BPC),
                                ].rearrange("p (x c) -> p x c", c=2),
                                VP2[:, c2 : c2 + 2]
                                .rearrange("p (x c) -> p x c", x=1)
                                .broadcast_to((128, NS // BPC // 2, 2)),
                            )
                    # in-place tanh over the whole group
                    nc.scalar.activation(
                        JT[:, :], JT[:, :], mybir.ActivationFunctionType.Tanh
                    )
                    for jj in range(L):
                        j = js[jj]
                        first = hc == 0 and g == 0 and jj == 0
                        last = hc == HC - 1 and g == len(groups) - 1 and jj == L - 1
                        nc.tensor.matmul(
                            ps_log[0:25, :],
                            wlz[:, hc * 625 + j * 25 : hc * 625 + (j + 1) * 25],
                            JT[:, jj * NS : (jj + 1) * NS],
                            start=first,
                            stop=last,
                            tile_position=(0, 0),
                            skip_group_check=True,
                        )
                        nc.tensor.matmul(
                            ps_log[32:57, :],
                            wlz[:, hc * 625 + j * 25 : hc * 625 + (j + 1) * 25],
                            JT[:, (L + jj) * NS : (L + jj + 1) * NS],
                            start=first,
                            stop=last,
                            tile_position=(0, 32),
                            skip_group_check=True,
                        )

            def proj_b():
                with tc.high_priority():
                    proj_phase(1, wqtB, wvtB)

            proj_phase(0, wqtA, wvtA)
            with tc.tile_pool(name="main", bufs=3) as mp:
                main_hc(0, mp, mid_cb=proj_b)
                main_hc(1, mp)
                main_hc(2, mp)
                main_hc(3, mp)

            # ---- masked softmax over k ----
            LG0 = pp.tile([25, NS], F32, name="LG0")
            LG1 = pp.tile([57, NS], F32, name="LG1")
            W_all = pp.tile([128, NS // 128, K], F32, name="W_all")
            nc.vector.tensor_copy(LG0[:, :], ps_log[0:25, :])
            nc.vector.tensor_copy(LG1[32:57, :], ps_log[32:57, :])
            for nsc in range(NS // 128):
                b = nsc // ((NS // BPC) // 128)
                LT = pp.tile([128, K], F32, name=f"LT{nsc}")
                for half in range(2):
                    ps_t = sps.tile([128, 25], F32, tag="ps_t", name="ps_t")
                    if half == 0:
                        src = LG0[0:25, nsc * 128 : (nsc + 1) * 128]
                        idn = ident[0:25, 0:25]
                    else:
                        src = LG1[32:57, nsc * 128 : (nsc + 1) * 128]
                        idn = ident[32:57, 32:57]
                    nc.tensor.transpose(ps_t[:, :], src, idn)
                    nc.vector.tensor_copy(
                        LT[:, half * 25 : (half + 1) * 25], ps_t[:, :]
                    )
                # masked = logits*maskf + (maskf-1)*1e9
                nc.vector.tensor_mul(
                    LT[:, :], LT[:, :], msk[:, b * K : (b + 1) * K]
                )
                nc.vector.tensor_add(
                    LT[:, :], LT[:, :], msk[:, KB + b * K : KB + (b + 1) * K]
                )
                mx = pp.tile([128, 1], F32, name=f"mx{nsc}")
                nc.vector.tensor_reduce(
                    mx[:, :], LT[:, :], axis=mybir.AxisListType.X,
                    op=mybir.AluOpType.max,
                )
                mxn = pp.tile([128, 1], F32, name=f"mxn{nsc}")
                nc.vector.tensor_scalar_mul(mxn[:, :], mx[:, :], -1.0)
                EX = pp.tile([128, K], F32, name=f"EX{nsc}")
                sm = pp.tile([128, 1], F32, name=f"sm{nsc}")
                nc.scalar.activation(
                    EX[:, :], LT[:, :], mybir.ActivationFunctionType.Exp,
                    bias=mxn[:, 0:1], accum_out=sm[:, 0:1],
                )
                rs = pp.tile([128, 1], F32, name=f"rs{nsc}")
                nc.vector.reciprocal(rs[:, :], sm[:, :])
                nc.vector.tensor_scalar_mul(
                    W_all[:, nsc, :], EX[:, :], rs[:, 0:1]
                )
            nc.sync.dma_start(
                out_h[:, :].rearrange("(c p) j -> p c j", p=128), W_all[:, :, :]
            )

    nc.finalize()
    return nc


def _prep_in_maps(v, q, box_mask, Wv, bv, Wq, bq, Wl):
    """Host-side layout prep: shard over B, transpose to device layouts."""
    import ml_dtypes

    v = np.asarray(v, np.float32).reshape(B, K, VD)
    q = np.asarray(q, np.float32).reshape(B, N * S, QD)
    mask = np.asarray(box_mask).astype(np.float32).reshape(B, K)

    WqT = np.asarray(Wq, np.float32).T                                # [QD, H]
    WvT = np.asarray(Wv, np.float32).T                                # [VD, H]
    WqTA = np.ascontiguousarray(WqT[:, :256]).astype(ml_dtypes.bfloat16)
    WqTB = np.ascontiguousarray(WqT[:, 256:]).astype(ml_dtypes.bfloat16)
    WvTA = np.ascontiguousarray(WvT[:, :256]).astype(ml_dtypes.bfloat16)
    WvTB = np.ascontiguousarray(WvT[:, 256:]).astype(ml_dtypes.bfloat16)
    wlb = np.zeros((128, 12), np.float32)
    wl_chunks = np.asarray(Wl, np.float32).reshape(4, 128).T          # [128, hc]
    wlb[:, 0:4] = wl_chunks
    wlb[:, 4:8] = np.asarray(bq, np.float32).reshape(4, 128).T
    wlb[:, 8:12] = np.asarray(bv, np.float32).reshape(4, 128).T
    # zero-padded Wl variants: wlz[p, hc*625 + j*25 + c] = Wl_chunk[p,hc]*(c==j)
    wlz = np.zeros((128, HC, 25, 25), np.float32)
    for j in range(25):
        wlz[:, :, j, j] = wl_chunks
    wlz = wlz.reshape(128, HC * 625).astype(ml_dtypes.bfloat16)
    ident = np.eye(128, dtype=np.float32)

    in_maps = []
    for c in range(NCORES):
        b0 = c * BPC
        qc = q[b0 : b0 + BPC].reshape(NS, QD)
        vc = v[b0 : b0 + BPC].reshape(KB, VD)
        qT = np.ascontiguousarray(qc.T).astype(ml_dtypes.bfloat16)    # [QD, NS]
        vT = np.ascontiguousarray(vc.T).astype(ml_dtypes.bfloat16)    # [VD, KB]
        mf = mask[b0 : b0 + BPC].reshape(1, KB)
        msk = np.zeros((128, 2 * KB), np.float32)
        msk[:, :KB] = mf
        msk[:, KB:] = (mf - 1.0) * 1e9
        in_maps.append(
            {
                "qT": qT,
                "vT": vT,
                "WqTA": WqTA,
                "WqTB": WqTB,
                "WvTA": WvTA,
                "WvTB": WvTB,
                "wlb": wlb,
                "wlz": wlz,
                "msk": msk,
                "ident": ident,
            }
        )
    return in_maps


def kernel(v, q, box_mask, tags_attention, Wv, bv, Wq, bq, Wl, bl):
    # bl shifts all unmasked logits uniformly -> cancels in softmax.
    # tags_attention is unused by the reference module.
    if "nc" not in _CACHE:
        _CACHE["nc"] = _build_nc()
    nc = _CACHE["nc"]
    in_maps = _prep_in_maps(v, q, box_mask, Wv, bv, Wq, bq, Wl)
    res = bass_utils.run_bass_kernel_spmd(
        nc,
        in_maps,
        core_ids=list(range(NCORES)),
        trace=bool(os.environ.get("KERNEL_TRACE")),
        tmpdir=os.environ.get("KERNEL_TMPDIR"),
    )
    _CACHE["last_result"] = res
    outs = [r["out"].reshape(BPC, N, S, K) for r in res.results]
    return np.concatenate(outs, axis=0)



# revision 2
# speedup vs baseline: 1.3502x; 1.3502x over previous
"""Fused additive-attention kernel for Trainium2 (8 NeuronCores, SPMD).

Computes  w = softmax_K( mask ? (Wl . tanh(vW_v^T + qW_q^T) + bl) : -1e9 )
without ever materializing the [B,N,S,K,H] joint_repr intermediate.

Sharding: data-parallel over batch B (16) across 8 cores -> 2 batches/core.
Weights replicated. Host does layout prep only (transposes / packing); all
FLOPs (matmuls, tanh, softmax) run on device.

Active-box packing: masked boxes contribute exactly 0 to the softmax, so the
host packs only the active boxes of each batch into Kpk = max_b(popcount)
slots (padded lanes get -1e9 logits via the mask trick) and scatters the
packed softmax back to K=50 positions afterwards.  All tanh/add/logit work
scales by Kpk/K.  The compiled kernel depends only on Kpk (cached; rebuilt
if an input's max active count changes) - it is correct for any box_mask.

Per-core dataflow (h on partitions for the hot loop), phased per h-chunk so
compute starts before all weights arrive:
  qpT[hc] [128(h), 512(b,ns)] = WqT-slice.T @ qT        (PE, psum acc, bf16)
  vp      [128(h), (b,i)] fp32 table (packed boxes)
  JT      [128, (j,strip,b,ns)] bf16 = tensor_scalar_add(QPs-slice, vp-col)
          (DVE tensor_scalar: 4x perf mode on bf16 SBUF; vp is the
          per-partition scalar operand)
  tanh in-place on JT (one big ACT op per (hc, j-group))
  logits  psum [32+Kh, 512]: rows 0:Kh = strip 0, rows 32:32+Kh = strip 1,
          accumulated with zero-padded Wl lhsT; the two strips share lhsT
          and run as adjacent matmuls (tile_position row offset 32).
  masked softmax over packed lanes after PE-transposing logits to [ns, i].
"""

import os
import sys

import numpy as np

sys.path.insert(0, "/opt/trn_rl_repo")

import concourse.bass as bass
import concourse.mybir as mybir
from concourse import bacc, bass_utils
from concourse.tile import TileContext

# Problem shapes (hardcoded per contract -- kernel.py must be self-contained)
B, N, S, K = 16, 4, 64, 50
VD, QD, H = 1024, 768, 512
NCORES = 8
BPC = B // NCORES          # batches per core = 2
NS = BPC * N * S           # 512 rows (b, n, s) per core
HC = H // 128              # 4 h-chunks
QC = QD // 128             # 6 qd-chunks
VC = VD // 128             # 8 vd-chunks
NSB = NS // BPC            # 256 (n,s) columns per batch

F32 = mybir.dt.float32
BF16 = mybir.dt.bfloat16

_CACHE = {}


def _groups(khalf, first_small):
    """Split range(khalf) into ~3 j-groups; small first group if requested."""
    if first_small:
        g0 = max(2, khalf // 5)
    else:
        g0 = (khalf + 2) // 3
    rest = khalf - g0
    g1 = (rest + 1) // 2
    sizes = [g0, g1, rest - g1]
    sizes = [s for s in sizes if s > 0]
    out, at = [], 0
    for s in sizes:
        out.append(list(range(at, at + s)))
        at += s
    return out


def _build_nc(kpk):
    kh = kpk // 2              # strip width (psum rows 0:kh and 32:32+kh)
    kb2 = BPC * kpk            # packed (b, i) columns per core

    nc = bacc.Bacc("TRN2", target_bir_lowering=False)

    qT_h = nc.dram_tensor("qT", [QD, NS], BF16, kind="ExternalInput")
    vT_h = nc.dram_tensor("vT", [VD, kb2], BF16, kind="ExternalInput")
    # weight slabs, pre-split by hc-pair: A = h cols 0:256, B = 256:512
    WqTA_h = nc.dram_tensor("WqTA", [QD, 256], BF16, kind="ExternalInput")
    WqTB_h = nc.dram_tensor("WqTB", [QD, 256], BF16, kind="ExternalInput")
    WvTA_h = nc.dram_tensor("WvTA", [VD, 256], BF16, kind="ExternalInput")
    WvTB_h = nc.dram_tensor("WvTB", [VD, 256], BF16, kind="ExternalInput")
    # packed [128, 12]: cols 0:4 Wl chunks, 4:8 bq chunks, 8:12 bv chunks
    wlb_h = nc.dram_tensor("wlb", [128, 12], F32, kind="ExternalInput")
    # zero-padded Wl variants: [128, hc*kh*kh + j*kh + c] = Wl[hc*128+p]*(c==j)
    wlz_h = nc.dram_tensor("wlz", [128, HC * kh * kh], BF16, kind="ExternalInput")
    # packed [128, 2*kb2]: cols 0:kb2 validf (b,i) replicated, then (validf-1)*1e9
    msk_h = nc.dram_tensor("msk", [128, 2 * kb2], F32, kind="ExternalInput")
    id_h = nc.dram_tensor("ident", [128, 128], F32, kind="ExternalInput")
    out_h = nc.dram_tensor("out", [NS, kpk], F32, kind="ExternalOutput")

    with TileContext(nc) as tc:
        with (
            tc.tile_pool(name="persist", bufs=1) as pp,
            tc.tile_pool(name="ppsum", bufs=1, space="PSUM") as ppsum,
            tc.tile_pool(name="smpsum", bufs=2, space="PSUM") as sps,
        ):
            # ---- DMA loads, chunked + ordered so the hc0 projection chain
            # starts after the first (qts, wqtA) chunk instead of all loads ----
            vts = pp.tile([128, VC, kb2], BF16, name="vts")
            nc.sync.dma_start(
                vts[:, :, :], vT_h[:, :].rearrange("(c p) j -> p c j", p=128)
            )
            qts = pp.tile([128, QC, NS], BF16, name="qts")
            wqtA = pp.tile([128, QC, 256], BF16, name="wqtA")
            qT_r = qT_h[:, :].rearrange("(c p) j -> p c j", p=128)
            wqA_r = WqTA_h[:, :].rearrange("(c p) j -> p c j", p=128)
            for c in range(3):
                nc.sync.dma_start(
                    qts[:, 2 * c : 2 * c + 2, :], qT_r[:, 2 * c : 2 * c + 2, :]
                )
                nc.sync.dma_start(
                    wqtA[:, 2 * c : 2 * c + 2, :], wqA_r[:, 2 * c : 2 * c + 2, :]
                )
                if c == 0:
                    wvtA = pp.tile([128, VC, 256], BF16, name="wvtA")
                    nc.sync.dma_start(
                        wvtA[:, :, :],
                        WvTA_h[:, :].rearrange("(c p) j -> p c j", p=128),
                    )
            wlb = pp.tile_from(wlb_h[:, :], name="wlb")
            wlz = pp.tile_from(wlz_h[:, :], name="wlz")
            msk = pp.tile_from(msk_h[:, :], name="msk")
            ident = pp.tile_from(id_h[:, :], name="ident")
            wqtB = pp.tile([128, QC, 256], BF16, name="wqtB")
            nc.sync.dma_start(
                wqtB[:, :, :], WqTB_h[:, :].rearrange("(c p) j -> p c j", p=128)
            )
            wvtB = pp.tile([128, VC, 256], BF16, name="wvtB")
            nc.sync.dma_start(
                wvtB[:, :, :], WvTB_h[:, :].rearrange("(c p) j -> p c j", p=128)
            )

            # qpT (all h-chunks): [128, (hc, b, ns)] bf16
            QPs = pp.tile([128, HC * NS], BF16, name="QPs")
            # packed v-proj table: [128, (hc, b, i)] fp32 (scalar operands)
            VP = pp.tile([128, HC * kb2], F32, name="VP")

            # logits psum: rows 0:kh <- strip 0, rows 32:32+kh <- strip 1
            ps_log = ppsum.tile([32 + kh, NS], F32, name="ps_log")

            def proj_phase(ph, wqt, wvt):
                """Compute QPs/VP h-chunks [2*ph, 2*ph+2) from slab wqt/wvt."""
                with tc.tile_pool(name=f"p1ps{ph}", bufs=2, space="PSUM") as p1ps:
                    for i in range(2):
                        hc = 2 * ph + i
                        pq = p1ps.tile([128, NS], F32, tag="pq", name="pq")
                        for qc in range(QC):
                            nc.tensor.matmul(
                                pq[:, :],
                                wqt[:, qc, i * 128 : (i + 1) * 128],
                                qts[:, qc, :],
                                start=(qc == 0),
                                stop=(qc == QC - 1),
                            )
                        pv = p1ps.tile([128, kb2], F32, tag="pv", name="pv")
                        for vc in range(VC):
                            nc.tensor.matmul(
                                pv[:, :],
                                wvt[:, vc, i * 128 : (i + 1) * 128],
                                vts[:, vc, :],
                                start=(vc == 0),
                                stop=(vc == VC - 1),
                            )
                        nc.vector.tensor_scalar_add(
                            QPs[:, hc * NS : (hc + 1) * NS],
                            pq[:, :],
                            wlb[:, HC + hc : HC + hc + 1],
                        )
                        nc.vector.tensor_scalar_add(
                            VP[:, hc * kb2 : (hc + 1) * kb2],
                            pv[:, :],
                            wlb[:, 2 * HC + hc : 2 * HC + hc + 1],
                        )

            groups_hc0 = _groups(kh, first_small=True)
            groups_hcx = _groups(kh, first_small=False)

            def main_hc(hc, mp, mid_cb=None):
                """Joint tanh + logit matmuls for one h-chunk."""
                groups = groups_hc0 if hc == 0 else groups_hcx
                for g, js in enumerate(groups):
                    if g == 1 and mid_cb is not None:
                        mid_cb()
                    L = len(js)
                    JT = mp.tile([128, 2 * L * NS], BF16, tag="JT", name="JT")
                    for jj, j in enumerate(js):
                        for strip in range(2):
                            slab = jj * 2 + strip
                            for b in range(BPC):
                                nc.vector.tensor_scalar_add(
                                    JT[:, slab * NS + b * NSB : slab * NS + (b + 1) * NSB],
                                    QPs[:, hc * NS + b * NSB : hc * NS + (b + 1) * NSB],
                                    VP[
                                        :,
                                        hc * kb2 + b * kpk + j + strip * kh
                                        : hc * kb2 + b * kpk + j + strip * kh + 1,
                                    ],
                                )
                    # in-place tanh over the whole group
                    nc.scalar.activation(
                        JT[:, :], JT[:, :], mybir.ActivationFunctionType.Tanh
                    )
                    for jj, j in enumerate(js):
                        first = hc == 0 and g == 0 and jj == 0
                        last = hc == HC - 1 and g == len(groups) - 1 and jj == L - 1
                        nc.tensor.matmul(
                            ps_log[0:kh, :],
                            wlz[:, hc * kh * kh + j * kh : hc * kh * kh + (j + 1) * kh],
                            JT[:, (jj * 2) * NS : (jj * 2 + 1) * NS],
                            start=first,
                            stop=last,
                            tile_position=(0, 0),
                            skip_group_check=True,
                        )
                        nc.tensor.matmul(
                            ps_log[32 : 32 + kh, :],
                            wlz[:, hc * kh * kh + j * kh : hc * kh * kh + (j + 1) * kh],
                            JT[:, (jj * 2 + 1) * NS : (jj * 2 + 2) * NS],
                            start=first,
                            stop=last,
                            tile_position=(0, 32),
                            skip_group_check=True,
                        )

            def proj_b():
                with tc.high_priority():
                    proj_phase(1, wqtB, wvtB)

            proj_phase(0, wqtA, wvtA)
            with tc.tile_pool(name="main", bufs=3) as mp:
                main_hc(0, mp, mid_cb=proj_b)
                main_hc(1, mp)
                main_hc(2, mp)
                main_hc(3, mp)

            # ---- masked softmax over packed lanes ----
            LG0 = pp.tile([kh, NS], F32, name="LG0")
            LG1 = pp.tile([32 + kh, NS], F32, name="LG1")
            W_all = pp.tile([128, NS // 128, kpk], F32, name="W_all")
            nc.vector.tensor_copy(LG0[:, :], ps_log[0:kh, :])
            nc.vector.tensor_copy(LG1[32 : 32 + kh, :], ps_log[32 : 32 + kh, :])
            for nsc in range(NS // 128):
                b = nsc // (NSB // 128)
                LT = pp.tile([128, kpk], F32, name=f"LT{nsc}")
                for half in range(2):
                    ps_t = sps.tile([128, kh], F32, tag="ps_t", name="ps_t")
                    if half == 0:
                        src = LG0[0:kh, nsc * 128 : (nsc + 1) * 128]
                        idn = ident[0:kh, 0:kh]
                    else:
                        src = LG1[32 : 32 + kh, nsc * 128 : (nsc + 1) * 128]
                        idn = ident[32 : 32 + kh, 32 : 32 + kh]
                    nc.tensor.transpose(ps_t[:, :], src, idn)
                    nc.vector.tensor_copy(
                        LT[:, half * kh : (half + 1) * kh], ps_t[:, :]
                    )
                # masked = logits*validf + (validf-1)*1e9
                nc.vector.tensor_mul(
                    LT[:, :], LT[:, :], msk[:, b * kpk : (b + 1) * kpk]
                )
                nc.vector.tensor_add(
                    LT[:, :], LT[:, :], msk[:, kb2 + b * kpk : kb2 + (b + 1) * kpk]
                )
                mx = pp.tile([128, 1], F32, name=f"mx{nsc}")
                nc.vector.tensor_reduce(
                    mx[:, :], LT[:, :], axis=mybir.AxisListType.X,
                    op=mybir.AluOpType.max,
                )
                mxn = pp.tile([128, 1], F32, name=f"mxn{nsc}")
                nc.vector.tensor_scalar_mul(mxn[:, :], mx[:, :], -1.0)
                EX = pp.tile([128, kpk], F32, name=f"EX{nsc}")
                sm = pp.tile([128, 1], F32, name=f"sm{nsc}")
                nc.scalar.activation(
                    EX[:, :], LT[:, :], mybir.ActivationFunctionType.Exp,
                    bias=mxn[:, 0:1], accum_out=sm[:, 0:1],
                )
                rs = pp.tile([128, 1], F32, name=f"rs{nsc}")
                nc.vector.reciprocal(rs[:, :], sm[:, :])
                nc.vector.tensor_scalar_mul(
                    W_all[:, nsc, :], EX[:, :], rs[:, 0:1]
                )
            nc.sync.dma_start(
                out_h[:, :].rearrange("(c p) j -> p c j", p=128), W_all[:, :, :]
            )

    nc.finalize()
    return nc


def _prep_in_maps(v, q, box_mask, Wv, bv, Wq, bq, Wl, kpk, active):
    """Host-side layout prep: shard over B, pack active boxes, transpose."""
    import ml_dtypes

    kh = kpk // 2
    kb2 = BPC * kpk

    v = np.asarray(v, np.float32).reshape(B, K, VD)
    q = np.asarray(q, np.float32).reshape(B, N * S, QD)

    # packed v + validity per batch
    vp = np.zeros((B, kpk, VD), np.float32)
    valid = np.zeros((B, kpk), np.float32)
    for b in range(B):
        kb = len(active[b])
        vp[b, :kb] = v[b, active[b]]
        valid[b, :kb] = 1.0

    WqT = np.asarray(Wq, np.float32).T                                # [QD, H]
    WvT = np.asarray(Wv, np.float32).T                                # [VD, H]
    WqTA = np.ascontiguousarray(WqT[:, :256]).astype(ml_dtypes.bfloat16)
    WqTB = np.ascontiguousarray(WqT[:, 256:]).astype(ml_dtypes.bfloat16)
    WvTA = np.ascontiguousarray(WvT[:, :256]).astype(ml_dtypes.bfloat16)
    WvTB = np.ascontiguousarray(WvT[:, 256:]).astype(ml_dtypes.bfloat16)
    wlb = np.zeros((128, 12), np.float32)
    wl_chunks = np.asarray(Wl, np.float32).reshape(4, 128).T          # [128, hc]
    wlb[:, 0:4] = wl_chunks
    wlb[:, 4:8] = np.asarray(bq, np.float32).reshape(4, 128).T
    wlb[:, 8:12] = np.asarray(bv, np.float32).reshape(4, 128).T
    # zero-padded Wl variants: wlz[p, hc*kh*kh + j*kh + c] = Wl_chunk[p,hc]*(c==j)
    wlz = np.zeros((128, HC, kh, kh), np.float32)
    for j in range(kh):
        wlz[:, :, j, j] = wl_chunks
    wlz = wlz.reshape(128, HC * kh * kh).astype(ml_dtypes.bfloat16)
    ident = np.eye(128, dtype=np.float32)

    in_maps = []
    for c in range(NCORES):
        b0 = c * BPC
        qc = q[b0 : b0 + BPC].reshape(NS, QD)
        vc = vp[b0 : b0 + BPC].reshape(kb2, VD)
        qT = np.ascontiguousarray(qc.T).astype(ml_dtypes.bfloat16)    # [QD, NS]
        vT = np.ascontiguousarray(vc.T).astype(ml_dtypes.bfloat16)    # [VD, kb2]
        mf = valid[b0 : b0 + BPC].reshape(1, kb2)
        msk = np.zeros((128, 2 * kb2), np.float32)
        msk[:, :kb2] = mf
        msk[:, kb2:] = (mf - 1.0) * 1e9
        in_maps.append(
            {
                "qT": qT,
                "vT": vT,
                "WqTA": WqTA,
                "WqTB": WqTB,
                "WvTA": WvTA,
                "WvTB": WvTB,
                "wlb": wlb,
                "wlz": wlz,
                "msk": msk,
                "ident": ident,
            }
        )
    return in_maps


def kernel(v, q, box_mask, tags_attention, Wv, bv, Wq, bq, Wl, bl):
    # bl shifts all unmasked logits uniformly -> cancels in softmax.
    # tags_attention is unused by the reference module.
    bm = np.asarray(box_mask).reshape(B, K)
    active = [np.nonzero(bm[b] > 0)[0] for b in range(B)]
    kmax = max(len(a) for a in active)
    if kmax == 0:
        # every box masked in every batch: reference softmax is uniform
        return np.full((B, N, S, K), 1.0 / K, np.float32)
    kpk = max(2, kmax + (kmax & 1))       # even, >= 2

    if _CACHE.get("kpk") != kpk:
        _CACHE["nc"] = _build_nc(kpk)
        _CACHE["kpk"] = kpk
    nc = _CACHE["nc"]
    in_maps = _prep_in_maps(v, q, box_mask, Wv, bv, Wq, bq, Wl, kpk, active)
    res = bass_utils.run_bass_kernel_spmd(
        nc,
        in_maps,
        core_ids=list(range(NCORES)),
        trace=bool(os.environ.get("KERNEL_TRACE")),
        tmpdir=os.environ.get("KERNEL_TMPDIR"),
    )
    _CACHE["last_result"] = res
    w = np.zeros((B, N, S, K), np.float32)
    for c in range(NCORES):
        wp = res.results[c]["out"].reshape(BPC, N, S, kpk)
        for bi in range(BPC):
            b = c * BPC + bi
            kb = len(active[b])
            if kb == 0:
                w[b] = 1.0 / K          # all-masked row: uniform softmax
            else:
                w[b][:, :, active[b]] = wp[bi][:, :, :kb]
    return w
